# revision 21
# baseline (speedup 1.0000x reference)
"""Trainium2 Bass kernel for DynamicTokenMixing (16-head attention, N=4096, C=1024).

Sharding: head-parallel across 8 NeuronCores, 2 heads per core. Each core
computes q/k/v projections for its 2 heads, full attention for those heads,
and a partial output projection; the host sums the 8 partials and adds bias.

v3: 16-bit matmuls everywhere (PE power-throttle caps sustained column issue;
fp32r modes are no faster than 16-bit under the cap, but 16-bit minimizes
total columns). Phase 2 is one software-pipelined stream over
(pair-of-strips, head, key-tile) steps with 1024-wide exp instructions.
Softmax normalization multiplies the staged (un-normalized) attention output
by a PE-broadcast reciprocal row (bf16), so the output projection runs as a
single K=128 matmul over both heads stacked; projection results DMA straight
from PSUM to DRAM. k-projections run first; q/v projections + v transposes
are interleaved into early phase-2 steps as deadline-gated closures.

Per-core dataflow:
  kT, qT    = (x @ Wk).T, (x @ Wq).T     [128 x 4096 fp16, dual-head stacked]
  vsb[j]    = v tiles in [token, (vA|1|vB|1)] layout (PE-transposed, bf16)
  st[j,i]   = sum_d k[j,d] q[i,d]        (scores, [key, query] layout, PSUM)
  et        = exp(0.5*st)                (bf16)
  av        = sum_j vsb[j]^T et[j,:]     (row 64 = denominator l)
  cat       = [stage_h0; stage_h1] * broadcast(1/l_h)   (bf16, [128 x 1024])
  out_tile  = cat^T @ [Wp_h0; Wp_h1]     (K=128; DMA from PSUM)
"""

import numpy as np
import ml_dtypes

import concourse.bass as bass
import concourse.mybir as mybir
import concourse.tile as tile
from concourse import bacc
from concourse.bass_utils import run_bass_kernel_spmd
from concourse.masks import make_identity

F32 = mybir.dt.float32
F16 = mybir.dt.float16
BF16 = mybir.dt.bfloat16

N = 4096          # tokens
C = 1024          # model dim
D = 64            # head dim
NHEADS = 16
GPD = 2
NCORES = 8
NCT = C // 128    # 8 contraction tiles
STRIP = 512
NSTRIP = N // STRIP          # 8
NPAIR = NSTRIP // 2          # 4 pairs of strips (1024 tokens each)
NJ = N // 128     # 32 key tiles
NBLK = NPAIR * 2  # 8 (pair, head) blocks
LAG = 5           # av lags st by LAG steps in the phase-2 pipeline
SCORE_SCALE = GPD * GPD * (D ** -0.5)  # 0.5


def build_nc(repeat=1, hw_loop=False):
    nc = bacc.Bacc("TRN2", target_bir_lowering=False, debug=False,
                   num_devices=NCORES)
    xt_d = nc.declare_dram_parameter("xt", [128, NCT, N], F16, isOutput=False)
    wq_d = nc.declare_dram_parameter("wq", [128, NCT, 128], F16, isOutput=False)
    wk_d = nc.declare_dram_parameter("wk", [128, NCT, 128], F16, isOutput=False)
    wv_d = nc.declare_dram_parameter("wv", [128, NCT, 128], F16, isOutput=False)
    wps_d = nc.declare_dram_parameter("wps", [128, C], BF16, isOutput=False)
    out_d = nc.declare_dram_parameter("out", [N, C], F32, isOutput=True)

    out_r = out_d[:].rearrange("(t p) o -> t p o", p=128)  # [32, 128, 1024]
    Exp = mybir.ActivationFunctionType.Exp
    mult = mybir.AluOpType.mult

    with tile.TileContext(nc) as tc:
        with (
            nc.allow_low_precision(reason="16-bit matmul inputs by design"),
            tc.tile_pool(name="persist", bufs=1) as persist,
        ):
            # resident x^T (fp16): 4 pair tiles of [128, 8, 2, 512]
            xt_sb = [persist.tile([128, NCT, 2, STRIP], F16, name=f"xt{p}")
                     for p in range(NPAIR)]
            wq_sb = persist.tile([128, NCT, 128], F16)
            wk_sb = persist.tile([128, NCT, 128], F16)
            wv_sb = persist.tile([128, NCT, 128], F16)
            wps_sb = persist.tile([128, C], BF16)
            qT = [persist.tile([128, 2, STRIP], F16, name=f"qT{p}")
                  for p in range(NPAIR)]
            kT = [persist.tile([128, 2, STRIP], F16, name=f"kT{p}")
                  for p in range(NPAIR)]
            vsb = persist.tile([128, NJ, 130], BF16)
            ident = persist.tile([128, 128], BF16)
            ones64 = persist.tile([1, 64], BF16)

            nc.sync.dma_start(xt_sb[0][:],
                              xt_d[:, :, 0:2 * STRIP]
                              .rearrange("p c (u s) -> p c u s", u=2))
            nc.sync.dma_start(wk_sb[:], wk_d[:])
            nc.sync.dma_start(wq_sb[:], wq_d[:])
            nc.sync.dma_start(wv_sb[:], wv_d[:])
            nc.sync.dma_start(wps_sb[:], wps_d[:])
            for p in range(1, NPAIR):
                nc.sync.dma_start(xt_sb[p][:],
                                  xt_d[:, :, bass.ts(p, 2 * STRIP)]
                                  .rearrange("p c (u s) -> p c u s", u=2))
            make_identity(nc, ident[:])
            nc.gpsimd.memset(ones64[:], 1.0)
            nc.gpsimd.memset(vsb[:, :, 64:65], 1.0)
            nc.gpsimd.memset(vsb[:, :, 129:130], 1.0)

            import contextlib
            rep_iter = ([None] if hw_loop and repeat > 1 else range(repeat))
            for _rep in rep_iter:
              with (tc.For_i(0, repeat, 1) if hw_loop and repeat > 1
                    else contextlib.nullcontext()):
                # ---- k-projections for all pairs (phase-2 scores need all
                # of kT up front); pair-0 q/v runs between k(0) and k(1) so
                # the PE has work while the later xt DMAs stream in ----
                def emit_k(p, k_pool):
                    k_ps = k_pool.tile([128, 2, STRIP], F32, tag="st",
                                       name=f"k_ps{p}")
                    for u in range(2):
                        for c in range(NCT):
                            nc.tensor.matmul(k_ps[:, u, :], wk_sb[:, c, :],
                                             xt_sb[p][:, c, u, :],
                                             start=(c == 0),
                                             stop=(c == NCT - 1))
                    nc.vector.tensor_copy(kT[p][:], k_ps[:])

                # ---- Phase 2 + interleaved q/v projections ----
                with (
                    tc.tile_pool(name="st_ps", bufs=2, space="PSUM") as st_pool,
                    tc.tile_pool(name="av_ps", bufs=1, space="PSUM") as av_pool,
                    tc.tile_pool(name="pp_ps", bufs=2, space="PSUM") as pp_pool,
                    tc.tile_pool(name="et_sb", bufs=LAG + 1) as et_pool,
                    tc.tile_pool(name="vt_sb", bufs=2) as vt_pool,
                    tc.tile_pool(name="cat_sb", bufs=2) as cat_pool,
                    tc.tile_pool(name="cm_sb", bufs=2) as cm_pool,
                    tc.tile_pool(name="tm_sb", bufs=2) as tm_pool,
                    tc.tile_pool(name="sl_sb", bufs=2) as sl_pool,
                    tc.tile_pool(name="rr_sb", bufs=3) as rr_pool,
                    tc.tile_pool(name="ob_sb", bufs=3) as ob_pool,
                ):
                    NSTEP = NBLK * NJ  # 256
                    ets = {}
                    avs = {}
                    cats = {}          # pair -> [stage_h0; stage_h1] bf16
                    cms = {}           # pair -> normalized cat
                    rrows = {}         # block -> bf16 reciprocal row
                    spread = []        # (min_step, closure), popped 1/step

                    def make_qv(p, u, which):
                        def qv():
                            if which == "q":
                                ps = pp_pool.tile([128, STRIP], F32, tag="pp",
                                                  name=f"qps{p}{u}")
                                for c in range(NCT):
                                    nc.tensor.matmul(ps[:], wq_sb[:, c, :],
                                                     xt_sb[p][:, c, u, :],
                                                     start=(c == 0),
                                                     stop=(c == NCT - 1))
                                nc.vector.tensor_copy(qT[p][:, u, :], ps[:])
                            else:
                                ps = pp_pool.tile([128, STRIP], F32, tag="pp",
                                                  name=f"vps{p}{u}")
                                for c in range(NCT):
                                    nc.tensor.matmul(ps[:], wv_sb[:, c, :],
                                                     xt_sb[p][:, c, u, :],
                                                     start=(c == 0),
                                                     stop=(c == NCT - 1))
                                vt = vt_pool.tile([128, STRIP], BF16, tag="vt")
                                nc.vector.tensor_copy(vt[:], ps[:])
                                for jj in range(4):
                                    j = 8 * p + 4 * u + jj
                                    tp = pp_pool.tile([128, 128], BF16,
                                                      tag="pp", name=f"tp{j}")
                                    nc.tensor.transpose(
                                        tp[:], vt[:, bass.ts(jj, 128)],
                                        ident[:])
                                    nc.vector.tensor_copy(vsb[:, j, 0:64],
                                                          tp[:, 0:64])
                                    nc.vector.tensor_copy(vsb[:, j, 65:129],
                                                          tp[:, 64:128])
                        return qv

                    def emit_st(n):
                        b, j = n // NJ, n % NJ
                        pair, h = b // 2, b % 2
                        hs = slice(64 * h, 64 * h + 64)
                        stt = st_pool.tile([128, 2, STRIP], F32, tag="st")
                        kt = kT[j // 8][hs, (j // 4) % 2, bass.ts(j % 4, 128)]
                        for u2 in range(2):
                            nc.tensor.matmul(stt[:, u2, :], kt,
                                             qT[pair][hs, u2, :],
                                             start=True, stop=True)
                        et = et_pool.tile([128, 2, STRIP], BF16, tag="et")
                        nc.scalar.activation(et[:], stt[:], Exp,
                                             scale=SCORE_SCALE)
                        ets[n] = et

                    def emit_av(n):
                        b, j = n // NJ, n % NJ
                        h = b % 2
                        if j == 0:
                            avs[b] = av_pool.tile([128, 2, STRIP], F32,
                                                  tag="av", name=f"av{b}")
                        av = avs[b]
                        et = ets.pop(n)
                        lhs = vsb[:, j, 65 * h:65 * h + 65]
                        for u2 in range(2):
                            nc.tensor.matmul(av[0:65, u2, :], lhs,
                                             et[:, u2, :],
                                             start=(j == 0), stop=(j == NJ - 1),
                                             skip_group_check=True)

                    def emit_block_tail(b):
                        # drains before the next block's av allocation (the
                        # av pool has a single rotating buffer)
                        av = avs.pop(b)
                        pair, h = b // 2, b % 2
                        sl = sl_pool.tile([1, 2, STRIP], F32, tag="sl")
                        nc.vector.tensor_copy(sl[:], av[64:65, :, :])
                        if h == 0:
                            cats[pair] = cat_pool.tile([128, 2, STRIP], BF16,
                                                       tag="cat",
                                                       name=f"cat{pair}")
                            nc.vector.tensor_copy(cats[pair][0:64, :, :],
                                                  av[0:64, :, :])
                        else:
                            tm = tm_pool.tile([64, 2, STRIP], BF16, tag="tm")
                            nc.vector.tensor_copy(tm[:], av[0:64, :, :])
                            nc.sync.dma_start(cats[pair][64:128, :, :], tm[:])
                        rr = rr_pool.tile([1, 2, STRIP], BF16, tag="rr")
                        nc.vector.reciprocal(rr[:, 0, :], sl[:, 0, :])
                        nc.vector.reciprocal(rr[:, 1, :], sl[:, 1, :])
                        rrows[b] = rr

                    def make_bc(pair):
                        def bc():
                            rr0 = rrows.pop(2 * pair)
                            rr1 = rrows.pop(2 * pair + 1)
                            cat = cats.pop(pair)
                            cm = cm_pool.tile([128, 2, STRIP], BF16, tag="cm",
                                              name=f"cm{pair}")
                            cms[pair] = cm
                            for u in range(2):
                                bcu = pp_pool.tile([128, STRIP], F32,
                                                   tag="pp", name=f"bc{u}")
                                nc.tensor.matmul(bcu[0:64, :], ones64[:],
                                                 rr0[0:1, u, :],
                                                 start=True, stop=True)
                                nc.tensor.matmul(bcu[64:128, :], ones64[:],
                                                 rr1[0:1, u, :],
                                                 start=True, stop=True)
                                nc.vector.tensor_mul(
                                    cm[:, u, :], cat[:, u, :], bcu[:])
                        return bc

                    def make_proj(pair, t, oc):
                        def proj():
                            cm = cms[pair]
                            osl = bass.ts(oc, STRIP)
                            pp = pp_pool.tile([128, STRIP], F32, tag="pp")
                            nc.tensor.matmul(pp[:],
                                             cm[:, t // 4, bass.ts(t % 4, 128)],
                                             wps_sb[:, osl],
                                             start=True, stop=True)
                            ob = ob_pool.tile([128, STRIP], F32, tag="ob")
                            nc.vector.tensor_copy(ob[:], pp[:])
                            nc.sync.dma_start(out_r[8 * pair + t][:, osl],
                                              ob[:])
                        return proj

                    # k(0), then pair-0 q/v (hides the xt(1..3) DMA wait);
                    # k(1..3) and the other pairs' q/v interleave into the
                    # stream as deadline-gated closures. k tiles share the
                    # st pool's buffers. st(n) for j>=8p reads kT[p], so
                    # k(p) must be EMITTED before step 8p - LAG margin.
                    emit_k(0, st_pool)
                    for u in range(2):
                        make_qv(0, u, "v")()
                    for u in range(2):
                        make_qv(0, u, "q")()
                    spread.append((0, lambda: emit_k(1, st_pool)))
                    spread.append((2, lambda: emit_k(2, st_pool)))
                    spread.append((4, make_qv(1, 0, "v")))
                    spread.append((6, lambda: emit_k(3, st_pool)))
                    # remaining pairs: deadline-gated closures inside phase 2
                    spread.append((9, make_qv(1, 1, "v")))
                    for p in range(2, NPAIR):
                        for u in range(2):
                            spread.append((8 * (p - 1) + 4 * u,
                                           make_qv(p, u, "v")))
                    for p in range(1, NPAIR):
                        spread.append((24 + 12 * p, make_qv(p, 0, "q")))
                        spread.append((28 + 12 * p, make_qv(p, 1, "q")))

                    for n in range(NSTEP + LAG):
                        if n < NSTEP:
                            emit_st(n)
                        if n >= LAG:
                            m = n - LAG
                            emit_av(m)
                            b, j = m // NJ, m % NJ
                            if j == NJ - 1:
                                emit_block_tail(b)
                                if b % 2 == 1:
                                    pair = b // 2
                                    base = 32 * (b + 1) + 8
                                    spread.append((base, make_bc(pair)))
                                    for t in range(8):
                                        for oc in range(2):
                                            spread.append(
                                                (base + 1 + 2 * t + oc,
                                                 make_proj(pair, t, oc)))
                            elif spread and spread[0][0] <= n:
                                spread.pop(0)[1]()
                    while spread:
                        spread.pop(0)[1]()
    nc.finalize()
    return nc


def _colq(h):
    return slice(h * D, (h + 1) * D)


def _colk(h):
    base = h * D if h < 8 else 2 * 512 + (h - 8) * D
    return slice(base, base + D)


def _colv(h):
    base = 512 + h * D if h < 8 else 3 * 512 + (h - 8) * D
    return slice(base, base + D)


def _pmajor(a):
    """[1024, m] -> [128, 8, m] with row index = t*128 + p."""
    m = a.shape[1]
    return np.ascontiguousarray(
        a.reshape(NCT, 128, m).transpose(1, 0, 2)).astype(np.float16)


def make_in_maps(x, Wq, Wkv, Wproj):
    x = np.asarray(x, np.float32).reshape(N, C)
    Wq = np.asarray(Wq, np.float32)
    Wkv = np.asarray(Wkv, np.float32)
    Wproj = np.asarray(Wproj, np.float32)
    xt = _pmajor(np.ascontiguousarray(x.T))  # [128, 8, 4096] fp16
    in_maps = []
    for core in range(NCORES):
        h0, h1 = 2 * core, 2 * core + 1
        in_maps.append({
            "xt": xt,
            "wq": _pmajor(np.concatenate(
                [Wq[:, _colq(h0)], Wq[:, _colq(h1)]], axis=1)),
            "wk": _pmajor(np.concatenate(
                [Wkv[:, _colk(h0)], Wkv[:, _colk(h1)]], axis=1)),
            "wv": _pmajor(np.concatenate(
                [Wkv[:, _colv(h0)], Wkv[:, _colv(h1)]], axis=1)),
            "wps": np.ascontiguousarray(np.concatenate(
                [Wproj[h0 * D:(h0 + 1) * D, :],
                 Wproj[h1 * D:(h1 + 1) * D, :]], axis=0)
            ).astype(ml_dtypes.bfloat16),
        })
    return in_maps


_NC = None


def _get_nc():
    global _NC
    if _NC is None:
        _NC = build_nc()
    return _NC


def run_spmd(in_maps, **kwargs):
    return run_bass_kernel_spmd(_get_nc(), in_maps, list(range(NCORES)), **kwargs)


def kernel(x, Wq, Wkv, Wproj, bproj, H=None, W=None, **_unused):
    in_maps = make_in_maps(x, Wq, Wkv, Wproj)
    res = run_spmd(in_maps)
    acc = np.zeros((N, C), np.float64)
    for r in res.results:
        acc += r["out"]
    out = acc.astype(np.float32) + np.asarray(bproj, np.float32)[None, :]
    return out.reshape(1, N, C)


if __name__ == "__main__":
    nc = build_nc()
    print("built ok")


# revision 24
# speedup vs baseline: 1.0092x; 1.0092x over previous
"""Trainium2 Bass kernel for DynamicTokenMixing (16-head attention, N=4096, C=1024).

Sharding: head-parallel across 8 NeuronCores, 2 heads per core. Each core
computes q/k/v projections for its 2 heads, full attention for those heads,
and a partial output projection; the host sums the 8 partials and adds bias.

v3: 16-bit matmuls everywhere (PE power-throttle caps sustained column issue;
fp32r modes are no faster than 16-bit under the cap, but 16-bit minimizes
total columns). Phase 2 is one software-pipelined stream over
(pair-of-strips, head, key-tile) steps with 1024-wide exp instructions.
Softmax normalization multiplies the staged (un-normalized) attention output
by a PE-broadcast reciprocal row (bf16), so the output projection runs as a
single K=128 matmul over both heads stacked; projection results DMA straight
from PSUM to DRAM. k-projections run first; q/v projections + v transposes
are interleaved into early phase-2 steps as deadline-gated closures.

Per-core dataflow:
  kT, qT    = (x @ Wk).T, (x @ Wq).T     [128 x 4096 fp16, dual-head stacked]
  vsb[j]    = v tiles in [token, (vA|1|vB|1)] layout (PE-transposed, bf16)
  st[j,i]   = sum_d k[j,d] q[i,d]        (scores, [key, query] layout, PSUM)
  et        = exp(0.5*st)                (bf16)
  av        = sum_j vsb[j]^T et[j,:]     (row 64 = denominator l)
  cat       = [stage_h0; stage_h1] * broadcast(1/l_h)   (bf16, [128 x 1024])
  out_tile  = cat^T @ [Wp_h0; Wp_h1]     (K=128; DMA from PSUM)
"""

import numpy as np
import ml_dtypes

import concourse.bass as bass
import concourse.mybir as mybir
import concourse.tile as tile
from concourse import bacc
from concourse.bass_utils import run_bass_kernel_spmd
from concourse.masks import make_identity

F32 = mybir.dt.float32
F16 = mybir.dt.float16
BF16 = mybir.dt.bfloat16

N = 4096          # tokens
C = 1024          # model dim
D = 64            # head dim
NHEADS = 16
GPD = 2
NCORES = 8
NCT = C // 128    # 8 contraction tiles
STRIP = 512
NSTRIP = N // STRIP          # 8
NPAIR = NSTRIP // 2          # 4 pairs of strips (1024 tokens each)
NJ = N // 128     # 32 key tiles
NBLK = NPAIR * 2  # 8 (pair, head) blocks
LAG = 5           # av lags st by LAG steps in the phase-2 pipeline
SCORE_SCALE = GPD * GPD * (D ** -0.5)  # 0.5


def build_nc(repeat=1, hw_loop=False):
    nc = bacc.Bacc("TRN2", target_bir_lowering=False, debug=False,
                   num_devices=NCORES)
    xt_d = nc.declare_dram_parameter("xt", [128, NCT, N], F16, isOutput=False)
    wq_d = nc.declare_dram_parameter("wq", [128, NCT, 128], F16, isOutput=False)
    wk_d = nc.declare_dram_parameter("wk", [128, NCT, 128], F16, isOutput=False)
    wv_d = nc.declare_dram_parameter("wv", [128, NCT, 128], F16, isOutput=False)
    wps_d = nc.declare_dram_parameter("wps", [128, C], BF16, isOutput=False)
    out_d = nc.declare_dram_parameter("out", [N, C], F32, isOutput=True)

    out_r = out_d[:].rearrange("(t p) o -> t p o", p=128)  # [32, 128, 1024]
    Exp = mybir.ActivationFunctionType.Exp
    mult = mybir.AluOpType.mult

    with tile.TileContext(nc) as tc:
        with (
            nc.allow_low_precision(reason="16-bit matmul inputs by design"),
            tc.tile_pool(name="persist", bufs=1) as persist,
        ):
            # resident x^T (fp16): 4 pair tiles of [128, 8, 2, 512]
            xt_sb = [persist.tile([128, NCT, 2, STRIP], F16, name=f"xt{p}")
                     for p in range(NPAIR)]
            wq_sb = persist.tile([128, NCT, 128], F16)
            wk_sb = persist.tile([128, NCT, 128], F16)
            wv_sb = persist.tile([128, NCT, 128], F16)
            wps_sb = persist.tile([128, C], BF16)
            qT = [persist.tile([128, 2, STRIP], F16, name=f"qT{p}")
                  for p in range(NPAIR)]
            kT = [persist.tile([128, 2, STRIP], F16, name=f"kT{p}")
                  for p in range(NPAIR)]
            vsb = persist.tile([128, NJ, 130], BF16)
            ident = persist.tile([128, 128], BF16)
            ones64 = persist.tile([1, 64], BF16)

            nc.sync.dma_start(xt_sb[0][:],
                              xt_d[:, :, 0:2 * STRIP]
                              .rearrange("p c (u s) -> p c u s", u=2))
            nc.sync.dma_start(wk_sb[:], wk_d[:])
            nc.sync.dma_start(wq_sb[:], wq_d[:])
            nc.sync.dma_start(wv_sb[:], wv_d[:])
            nc.sync.dma_start(wps_sb[:], wps_d[:])
            for p in range(1, NPAIR):
                nc.scalar.dma_start(xt_sb[p][:],
                                    xt_d[:, :, bass.ts(p, 2 * STRIP)]
                                    .rearrange("p c (u s) -> p c u s", u=2))
            make_identity(nc, ident[:])
            nc.gpsimd.memset(ones64[:], 1.0)
            nc.gpsimd.memset(vsb[:, :, 64:65], 1.0)
            nc.gpsimd.memset(vsb[:, :, 129:130], 1.0)

            import contextlib
            rep_iter = ([None] if hw_loop and repeat > 1 else range(repeat))
            for _rep in rep_iter:
              with (tc.For_i(0, repeat, 1) if hw_loop and repeat > 1
                    else contextlib.nullcontext()):
                # ---- k-projections for all pairs (phase-2 scores need all
                # of kT up front); pair-0 q/v runs between k(0) and k(1) so
                # the PE has work while the later xt DMAs stream in ----
                def emit_k(p, k_pool):
                    k_ps = k_pool.tile([128, 2, STRIP], F32, tag="st",
                                       name=f"k_ps{p}")
                    for u in range(2):
                        for c in range(NCT):
                            nc.tensor.matmul(k_ps[:, u, :], wk_sb[:, c, :],
                                             xt_sb[p][:, c, u, :],
                                             start=(c == 0),
                                             stop=(c == NCT - 1))
                    nc.vector.tensor_copy(kT[p][:], k_ps[:])

                # ---- Phase 2 + interleaved q/v projections ----
                with (
                    tc.tile_pool(name="st_ps", bufs=2, space="PSUM") as st_pool,
                    tc.tile_pool(name="av_ps", bufs=1, space="PSUM") as av_pool,
                    tc.tile_pool(name="pp_ps", bufs=2, space="PSUM") as pp_pool,
                    tc.tile_pool(name="et_sb", bufs=LAG + 1) as et_pool,
                    tc.tile_pool(name="vt_sb", bufs=2) as vt_pool,
                    tc.tile_pool(name="cat_sb", bufs=2) as cat_pool,
                    tc.tile_pool(name="cm_sb", bufs=2) as cm_pool,
                    tc.tile_pool(name="tm_sb", bufs=2) as tm_pool,
                    tc.tile_pool(name="sl_sb", bufs=2) as sl_pool,
                    tc.tile_pool(name="rr_sb", bufs=3) as rr_pool,
                    tc.tile_pool(name="ob_sb", bufs=3) as ob_pool,
                ):
                    NSTEP = NBLK * NJ  # 256
                    ets = {}
                    avs = {}
                    cats = {}          # pair -> [stage_h0; stage_h1] bf16
                    cms = {}           # pair -> normalized cat
                    rrows = {}         # block -> bf16 reciprocal row
                    spread = []        # (min_step, closure), popped 1/step

                    def make_qv(p, u, which):
                        def qv():
                            if which == "q":
                                ps = pp_pool.tile([128, STRIP], F32, tag="pp",
                                                  name=f"qps{p}{u}")
                                for c in range(NCT):
                                    nc.tensor.matmul(ps[:], wq_sb[:, c, :],
                                                     xt_sb[p][:, c, u, :],
                                                     start=(c == 0),
                                                     stop=(c == NCT - 1))
                                nc.vector.tensor_copy(qT[p][:, u, :], ps[:])
                            else:
                                ps = pp_pool.tile([128, STRIP], F32, tag="pp",
                                                  name=f"vps{p}{u}")
                                for c in range(NCT):
                                    nc.tensor.matmul(ps[:], wv_sb[:, c, :],
                                                     xt_sb[p][:, c, u, :],
                                                     start=(c == 0),
                                                     stop=(c == NCT - 1))
                                vt = vt_pool.tile([128, STRIP], BF16, tag="vt")
                                nc.vector.tensor_copy(vt[:], ps[:])
                                for jj in range(4):
                                    j = 8 * p + 4 * u + jj
                                    tp = pp_pool.tile([128, 128], BF16,
                                                      tag="pp", name=f"tp{j}")
                                    nc.tensor.transpose(
                                        tp[:], vt[:, bass.ts(jj, 128)],
                                        ident[:])
                                    nc.vector.tensor_copy(vsb[:, j, 0:64],
                                                          tp[:, 0:64])
                                    nc.vector.tensor_copy(vsb[:, j, 65:129],
                                                          tp[:, 64:128])
                        return qv

                    def emit_st(n):
                        b, j = n // NJ, n % NJ
                        pair, h = b // 2, b % 2
                        hs = slice(64 * h, 64 * h + 64)
                        stt = st_pool.tile([128, 2, STRIP], F32, tag="st")
                        kt = kT[j // 8][hs, (j // 4) % 2, bass.ts(j % 4, 128)]
                        for u2 in range(2):
                            nc.tensor.matmul(stt[:, u2, :], kt,
                                             qT[pair][hs, u2, :],
                                             start=True, stop=True)
                        et = et_pool.tile([128, 2, STRIP], BF16, tag="et")
                        nc.scalar.activation(et[:], stt[:], Exp,
                                             scale=SCORE_SCALE)
                        ets[n] = et

                    def emit_av(n):
                        b, j = n // NJ, n % NJ
                        h = b % 2
                        if j == 0:
                            avs[b] = av_pool.tile([128, 2, STRIP], F32,
                                                  tag="av", name=f"av{b}")
                        av = avs[b]
                        et = ets.pop(n)
                        lhs = vsb[:, j, 65 * h:65 * h + 65]
                        for u2 in range(2):
                            nc.tensor.matmul(av[0:65, u2, :], lhs,
                                             et[:, u2, :],
                                             start=(j == 0), stop=(j == NJ - 1),
                                             skip_group_check=True)

                    def emit_block_tail(b):
                        # drains before the next block's av allocation (the
                        # av pool has a single rotating buffer)
                        av = avs.pop(b)
                        pair, h = b // 2, b % 2
                        sl = sl_pool.tile([1, 2, STRIP], F32, tag="sl")
                        nc.vector.tensor_copy(sl[:], av[64:65, :, :])
                        if h == 0:
                            cats[pair] = cat_pool.tile([128, 2, STRIP], BF16,
                                                       tag="cat",
                                                       name=f"cat{pair}")
                            nc.vector.tensor_copy(cats[pair][0:64, :, :],
                                                  av[0:64, :, :])
                        else:
                            tm = tm_pool.tile([64, 2, STRIP], BF16, tag="tm")
                            nc.vector.tensor_copy(tm[:], av[0:64, :, :])
                            nc.sync.dma_start(cats[pair][64:128, :, :], tm[:])
                        rr = rr_pool.tile([1, 2, STRIP], BF16, tag="rr")
                        nc.vector.reciprocal(rr[:, 0, :], sl[:, 0, :])
                        nc.vector.reciprocal(rr[:, 1, :], sl[:, 1, :])
                        rrows[b] = rr

                    def make_bc(pair):
                        def bc():
                            rr0 = rrows.pop(2 * pair)
                            rr1 = rrows.pop(2 * pair + 1)
                            cat = cats.pop(pair)
                            cm = cm_pool.tile([128, 2, STRIP], BF16, tag="cm",
                                              name=f"cm{pair}")
                            cms[pair] = cm
                            for u in range(2):
                                bcu = pp_pool.tile([128, STRIP], F32,
                                                   tag="pp", name=f"bc{u}")
                                nc.tensor.matmul(bcu[0:64, :], ones64[:],
                                                 rr0[0:1, u, :],
                                                 start=True, stop=True)
                                nc.tensor.matmul(bcu[64:128, :], ones64[:],
                                                 rr1[0:1, u, :],
                                                 start=True, stop=True)
                                nc.vector.tensor_mul(
                                    cm[:, u, :], cat[:, u, :], bcu[:])
                        return bc

                    def make_proj(pair, t, oc):
                        def proj():
                            cm = cms[pair]
                            osl = bass.ts(oc, STRIP)
                            last = (pair == NPAIR - 1)
                            # final pair: av pool is free, use it to deepen
                            # the pp rotation; bounce via the idle ACT engine
                            if last and (2 * t + oc) % 2 == 1:
                                pp = av_pool.tile([128, 2, STRIP], F32,
                                                  tag="av", name="ppav")
                                pp = pp[:, 0, :]
                            else:
                                pp = pp_pool.tile([128, STRIP], F32, tag="pp")
                            nc.tensor.matmul(pp,
                                             cm[:, t // 4, bass.ts(t % 4, 128)],
                                             wps_sb[:, osl],
                                             start=True, stop=True)
                            ob = ob_pool.tile([128, STRIP], F32, tag="ob")
                            if last:
                                nc.scalar.copy(ob[:], pp)
                            else:
                                nc.vector.tensor_copy(ob[:], pp)
                            nc.sync.dma_start(out_r[8 * pair + t][:, osl],
                                              ob[:])
                        return proj

                    # PE warmup during the initial DMA wait: ramps the PE
                    # p-state so k0/qv0 run at full clock
                    warm = st_pool.tile([128, 2, STRIP], F32, tag="st",
                                        name="warm")
                    for _ in range(24):
                        nc.tensor.matmul(warm[:, 0, 0:128], ident[:],
                                         ident[:], start=True, stop=True)
                    # k(0), then pair-0 q/v (hides the xt(1..3) DMA wait);
                    # k(1..3) and the other pairs' q/v interleave into the
                    # stream as deadline-gated closures. k tiles share the
                    # st pool's buffers. st(n) for j>=8p reads kT[p], so
                    # k(p) must be EMITTED before step 8p.
                    emit_k(0, st_pool)
                    make_qv(0, 0, "v")()
                    for u in range(2):
                        make_qv(0, u, "q")()
                    spread.append((0, make_qv(0, 1, "v")))
                    spread.append((0, lambda: emit_k(1, st_pool)))
                    spread.append((2, lambda: emit_k(2, st_pool)))
                    spread.append((4, make_qv(1, 0, "v")))
                    spread.append((6, lambda: emit_k(3, st_pool)))
                    # remaining pairs: deadline-gated closures inside phase 2
                    spread.append((9, make_qv(1, 1, "v")))
                    for p in range(2, NPAIR):
                        for u in range(2):
                            spread.append((8 * (p - 1) + 4 * u,
                                           make_qv(p, u, "v")))
                    for p in range(1, NPAIR):
                        spread.append((24 + 12 * p, make_qv(p, 0, "q")))
                        spread.append((28 + 12 * p, make_qv(p, 1, "q")))

                    for n in range(NSTEP + LAG):
                        if n < NSTEP:
                            emit_st(n)
                        if n >= LAG:
                            m = n - LAG
                            emit_av(m)
                            b, j = m // NJ, m % NJ
                            if j == NJ - 1:
                                emit_block_tail(b)
                                if b % 2 == 1:
                                    pair = b // 2
                                    base = 32 * (b + 1) + 8
                                    spread.append((base, make_bc(pair)))
                                    for t in range(8):
                                        for oc in range(2):
                                            spread.append(
                                                (base + 1 + 2 * t + oc,
                                                 make_proj(pair, t, oc)))
                            elif spread and spread[0][0] <= n:
                                spread.pop(0)[1]()
                    while spread:
                        spread.pop(0)[1]()
    nc.finalize()
    return nc


def _colq(h):
    return slice(h * D, (h + 1) * D)


def _colk(h):
    base = h * D if h < 8 else 2 * 512 + (h - 8) * D
    return slice(base, base + D)


def _colv(h):
    base = 512 + h * D if h < 8 else 3 * 512 + (h - 8) * D
    return slice(base, base + D)


def _pmajor(a):
    """[1024, m] -> [128, 8, m] with row index = t*128 + p."""
    m = a.shape[1]
    return np.ascontiguousarray(
        a.reshape(NCT, 128, m).transpose(1, 0, 2)).astype(np.float16)


def make_in_maps(x, Wq, Wkv, Wproj):
    x = np.asarray(x, np.float32).reshape(N, C)
    Wq = np.asarray(Wq, np.float32)
    Wkv = np.asarray(Wkv, np.float32)
    Wproj = np.asarray(Wproj, np.float32)
    xt = _pmajor(np.ascontiguousarray(x.T))  # [128, 8, 4096] fp16
    in_maps = []
    for core in range(NCORES):
        h0, h1 = 2 * core, 2 * core + 1
        in_maps.append({
            "xt": xt,
            "wq": _pmajor(np.concatenate(
                [Wq[:, _colq(h0)], Wq[:, _colq(h1)]], axis=1)),
            "wk": _pmajor(np.concatenate(
                [Wkv[:, _colk(h0)], Wkv[:, _colk(h1)]], axis=1)),
            "wv": _pmajor(np.concatenate(
                [Wkv[:, _colv(h0)], Wkv[:, _colv(h1)]], axis=1)),
            "wps": np.ascontiguousarray(np.concatenate(
                [Wproj[h0 * D:(h0 + 1) * D, :],
                 Wproj[h1 * D:(h1 + 1) * D, :]], axis=0)
            ).astype(ml_dtypes.bfloat16),
        })
    return in_maps


_NC = None


def _get_nc():
    global _NC
    if _NC is None:
        _NC = build_nc()
    return _NC


def run_spmd(in_maps, **kwargs):
    return run_bass_kernel_spmd(_get_nc(), in_maps, list(range(NCORES)), **kwargs)


def kernel(x, Wq, Wkv, Wproj, bproj, H=None, W=None, **_unused):
    in_maps = make_in_maps(x, Wq, Wkv, Wproj)
    res = run_spmd(in_maps)
    acc = np.zeros((N, C), np.float64)
    for r in res.results:
        acc += r["out"]
    out = acc.astype(np.float32) + np.asarray(bproj, np.float32)[None, :]
    return out.reshape(1, N, C)


if __name__ == "__main__":
    nc = build_nc()
    print("built ok")
